# revision 7
# baseline (speedup 1.0000x reference)
"""Trainium2 Bass kernel for dense_cnn problem.

Math (per batch element n, C=128 channels, H=W=56, G=8):
  t1 = conv_h(x, w1)          5-tap conv over H with full channel mixing
  t3 = dwconv_h(t1, w3)       3-tap depthwise conv over H
  t4[g] = sum_{c,k} x[c, h, w+2k-2] * w4[c,k,g]   (3 width taps, dil 2)
  out[c] = t3[c] * t4[c % 8]

Device strategy (data-parallel, 4 batch elems per core across 8 cores):
  - Fold t3 = w3 (*) w1 (*) x into ONE 7-tap H-conv with combined weights
    wc[f, ci, co] = sum_{d+e=f} w3[co,d] * w1[co,ci,e]  -> PE matmuls.
    The fold is only exact where the intermediate t1 index stays in
    [0,56); 4 tiny correction matmuls fix output rows 0 and 55.
  - t4 via ONE packed matmul per chunk producing y[32k+g] = sum_c
    x[c]*w4[c,k,g] at partition groups {0-7, 32-39, 64-71}, an ACT copy
    of y to SBUF, then THREE CONCURRENT row-tiled selector matmuls
    (tile_position=(32k,0)) that apply the +/-2 W-shifts and broadcast
    g -> 128 channels while accumulating in one PSUM bank.  Streams
    1x448 + 3x448-concurrent rows instead of 3x448 serial.
  - All matmuls in fp16 (1 cycle/row like bf16 but 10-bit mantissa:
    rel err ~4e-4 vs 2.6e-3); accumulation fp32 in PSUM.
  - Output DMA'd as fp16 (halves write traffic; host upcasts to fp32).
  - Per 8-row chunk: psA (t3), psY (y), psB (t4 broadcast); ScalarE
    copies psY->SBUF and psB->SBUF, VectorE multiplies psA*t4s -> fp16.
  - Startup: GpSimd memsets a dummy tile early; 8 dummy matmuls trip
    the PE_HAM clock gate (1.2 -> 2.4 GHz) while the first DMAs stream;
    x elem 0 lands chunk-first so real matmuls start ~10.3us.
"""

import sys

sys.path.insert(0, "/opt/trn_rl_repo")

import ml_dtypes
import numpy as np

import concourse.bacc as bacc
import concourse.bass as bass
import concourse.mybir as mybir
import concourse.tile as tile
from concourse import bass_utils

N, C, H, W, G = 32, 128, 56, 56, 8
NCORES = 8
NPC = N // NCORES  # batch elems per core
CH = 8             # H rows per chunk
NCHUNK = H // CH

F32 = mybir.dt.float32
F16 = mybir.dt.float16

T4_PACKED = False   # y-matmul + row-tiled broadcast (False: 3 plain taps)
NDUMMY = 8

TRACE = False
TRACE_DIR = None
LAST_EXEC_NS = None
LAST_RESULTS = None

_COMPILED = None


def _enable_trace_hook():
    """The agent image's ``antenv`` lacks ``axon_hooks``, so the boot-time
    NTFF hook registration silently degraded. Recreate the module and
    register the same ctypes-based hook; also skip the bucket upload."""
    import sys as _sys
    import types

    if "antenv.axon_hooks" not in _sys.modules:
        mod = types.ModuleType("antenv.axon_hooks")
        mod._hook = None

        def set_axon_ntff_profile_hook(h):
            mod._hook = h

        def get_axon_ntff_profile_hook():
            return mod._hook

        mod.set_axon_ntff_profile_hook = set_axon_ntff_profile_hook
        mod.get_axon_ntff_profile_hook = get_axon_ntff_profile_hook
        _sys.modules["antenv.axon_hooks"] = mod
        import antenv

        antenv.axon_hooks = mod

    from antenv.axon_hooks import get_axon_ntff_profile_hook as _get

    if _get() is None:
        from trn_agent_boot.trn_boot import _ntff_profile_via_ctypes

        hook = _ntff_profile_via_ctypes("/opt/axon/libaxon_pjrt.so")
        if hook is not None:
            _sys.modules["antenv.axon_hooks"].set_axon_ntff_profile_hook(hook)

    bass_utils.upload_artifacts = lambda tmpdir: f"local:{tmpdir}"


def _t3_matmuls(c, pa, xc, wc_t):
    """(lhsT, rhs, out) list accumulating the folded 7-tap conv for the
    8-row chunk c, with row clipping at the H borders plus the t1-clip
    correction taps. Output row o of the chunk reads x row 8c+o+f-3."""
    h0 = c * CH
    mms = []
    # f=3 covers the full chunk for every c -> emitted first (start=True)
    for f in (3, 0, 1, 2, 4, 5, 6):
        o_lo = max(0, 3 - f - h0)
        o_hi = min(CH, H + 3 - f - h0)
        if o_lo >= o_hi:
            continue
        r0 = h0 + o_lo + f - 3
        r1 = h0 + o_hi + f - 3
        mms.append((wc_t[:, f, :], xc[:, r0:r1, :], pa[:, o_lo:o_hi, :]))
    if c == 0:
        # fold wrongly includes t1[-1] at h=0: subtract w3[0]*w1[e]*x[e-3]
        for j in range(2):
            mms.append((wc_t[:, 7 + j, :], xc[:, j : j + 1, :], pa[:, 0:1, :]))
    if c == NCHUNK - 1:
        # fold wrongly includes t1[56] at h=55
        for j in range(2):
            mms.append(
                (wc_t[:, 9 + j, :], xc[:, 54 + j : 55 + j, :], pa[:, CH - 1 : CH, :])
            )
    return mms


def _t4_matmuls(c, pb, xc, w4_t):
    """Fallback t4: 3 width taps at offsets -2/0/+2, col-clipped."""
    h0 = c * CH
    rows = xc[:, h0 : h0 + CH, :]
    return [
        (w4_t[:, 1, :], rows, pb[:]),                                     # 0
        (w4_t[:, 0, :], xc[:, h0 : h0 + CH, 0 : W - 2], pb[:, :, 2:W]),   # -2
        (w4_t[:, 2, :], xc[:, h0 : h0 + CH, 2:W], pb[:, :, 0 : W - 2]),   # +2
    ]


def _build():
    nc = bacc.Bacc(
        "TRN2",
        target_bir_lowering=False,
        debug=False,
        enable_asserts=False,
        num_devices=NCORES,
    )

    x_d = nc.dram_tensor("x_s", (NPC, C, H, W), F16, kind="ExternalInput").ap()
    wc_d = nc.dram_tensor("wc", (C, 11, C), F16, kind="ExternalInput").ap()
    w4_shape = (C, 128) if T4_PACKED else (C, 3, C)
    w4_d = nc.dram_tensor("w4y", w4_shape, F16, kind="ExternalInput").ap()
    sel_d = nc.dram_tensor("sel", (128, C), F16, kind="ExternalInput").ap()
    out_d = nc.dram_tensor("out", (NPC, C, H, W), F16, kind="ExternalOutput").ap()

    with tile.TileContext(nc) as tc:
        with (
            tc.tile_pool(name="wpool", bufs=1) as wpool,
            tc.tile_pool(name="xpool", bufs=1) as xpool,
            tc.tile_pool(name="ypool", bufs=2) as ypool,
            tc.tile_pool(name="t4pool", bufs=3) as t4pool,
            tc.tile_pool(name="opool", bufs=3) as opool,
            tc.tile_pool(name="psA", bufs=3, space="PSUM") as papool,
            tc.tile_pool(name="psB", bufs=2, space="PSUM") as pbpool,
            tc.tile_pool(name="psY", bufs=2, space="PSUM") as pypool,
            tc.tile_pool(name="psD", bufs=1, space="PSUM") as pdpool,
        ):
            # Dummy matmuls while the first DMAs stream in: PE_HAM ungates
            # the 2.4 GHz clock only after ~3.4us of sustained activity.
            # GpSimd does the memset (its preamble finishes earliest).
            dmy = wpool.tile([C, 512], F16)
            nc.gpsimd.memset(dmy[:], 0.0)
            dps = pdpool.tile([C, 512], F32)
            for _ in range(NDUMMY):
                nc.tensor.matmul(
                    dps[:], lhsT=dmy[:, 0:C], rhs=dmy[:], start=True, stop=True
                )

            xcs = []
            for n in range(NPC):
                xc = xpool.tile([C, H, W], F16, name=f"xc{n}")
                xcs.append(xc)

            wc_t = wpool.tile([C, 11, C], F16)
            w4_t = wpool.tile(list(w4_shape), F16)
            sel_t = wpool.tile([128, C], F16)

            # first batch elem lands chunk 0 first so real matmuls can
            # start as soon as possible; weights go right behind it
            nc.sync.dma_start(xcs[0][:, 0:CH, :], x_d[0, :, 0:CH, :])
            nc.sync.dma_start(wc_t[:], wc_d[:])
            nc.sync.dma_start(w4_t[:], w4_d[:])
            nc.sync.dma_start(sel_t[:], sel_d[:])
            nc.sync.dma_start(xcs[0][:, CH:H, :], x_d[0, :, CH:H, :])
            for n in range(1, NPC):
                nc.sync.dma_start(xcs[n][:], x_d[n])

            for n in range(NPC):
                xc = xcs[n]
                ot = None

                for c in range(NCHUNK):
                    h0 = c * CH

                    if T4_PACKED:
                        # y[32k+g, h, w] = sum_c x[c,h,w] w4[c,k,g]
                        py = pypool.tile([C, CH, W], F32)
                        nc.tensor.matmul(
                            py[:], lhsT=w4_t[:], rhs=xc[:, h0 : h0 + CH, :],
                            start=True, stop=True,
                        )

                    pa = papool.tile([C, CH, W], F32)
                    mms = _t3_matmuls(c, pa, xc, wc_t)
                    for i, (lhsT, rhs, outap) in enumerate(mms):
                        nc.tensor.matmul(
                            outap,
                            lhsT=lhsT,
                            rhs=rhs,
                            start=(i == 0),
                            stop=(i == len(mms) - 1),
                        )

                    pb = pbpool.tile([C, CH, W], F32)
                    if T4_PACKED:
                        ys = ypool.tile([C, CH, W], F16)
                        nc.scalar.copy(ys[0:72, :, :], py[0:72, :, :])
                        # three concurrent row-tiled selector matmuls:
                        # tile k contracts y partitions 32k..32k+7 and
                        # broadcasts g -> all 128 channels with W-shift
                        nc.tensor.matmul(
                            pb[:], lhsT=sel_t[32:40, :], rhs=ys[32:40, :, :],
                            start=True, stop=False, tile_position=(32, 0),
                        )
                        nc.tensor.matmul(
                            pb[:, :, 2:W], lhsT=sel_t[0:8, :],
                            rhs=ys[0:8, :, 0 : W - 2],
                            start=False, stop=False, tile_position=(0, 0),
                        )
                        nc.tensor.matmul(
                            pb[:, :, 0 : W - 2], lhsT=sel_t[64:72, :],
                            rhs=ys[64:72, :, 2:W],
                            start=False, stop=True, tile_position=(64, 0),
                        )
                    else:
                        mmsb = _t4_matmuls(c, pb, xc, w4_t)
                        for i, (lhsT, rhs, outap) in enumerate(mmsb):
                            nc.tensor.matmul(
                                outap,
                                lhsT=lhsT,
                                rhs=rhs,
                                start=(i == 0),
                                stop=(i == len(mmsb) - 1),
                            )

                    t4s = t4pool.tile([C, CH, W], F16)
                    nc.scalar.copy(t4s[:], pb[:])

                    # output tiles: chunks [0..3], [4..5], [6] per elem ->
                    # 3 DMA issues; the last piece is small so the tail
                    # after the final matmul is short
                    if c == 0:
                        ot = opool.tile([C, 4 * CH, W], F16)
                        ot_c0 = 0
                    elif c == 4:
                        ot = opool.tile([C, 2 * CH, W], F16)
                        ot_c0 = 4
                    elif c == 6:
                        ot = opool.tile([C, CH, W], F16)
                        ot_c0 = 6
                    nc.vector.tensor_mul(
                        ot[:, (c - ot_c0) * CH : (c - ot_c0 + 1) * CH, :],
                        pa[:], t4s[:],
                    )
                    if c in (3, 5, 6):
                        rows = (c - ot_c0 + 1) * CH
                        nc.sync.dma_start(
                            out_d[n, :, ot_c0 * CH : ot_c0 * CH + rows, :],
                            ot[:, 0:rows, :],
                        )

    nc.compile()
    return nc


def _get_compiled():
    global _COMPILED
    if _COMPILED is None:
        _COMPILED = _build()
    return _COMPILED


def _prep_weights(w1, w3, w4):
    w1c = np.asarray(w1, dtype=np.float32)[:, :, :, 0]  # (co, ci, 5)
    w3c = np.asarray(w3, dtype=np.float32)[:, 0, :, 0]  # (co, 3)
    wc = np.zeros((C, 11, C), dtype=np.float32)         # (ci, tap, co)
    for d in range(3):
        for e in range(5):
            # wc[ci, d+e, co] += w1[co, ci, e] * w3[co, d]
            wc[:, d + e, :] += (w1c[:, :, e] * w3c[:, d][:, None]).T
    # border clip corrections (see _t3_matmuls): taps 7,8 fix h=0; 9,10 h=55
    for j, e in enumerate((3, 4)):
        wc[:, 7 + j, :] = -(w1c[:, :, e] * w3c[:, 0][:, None]).T
    for j, e in enumerate((0, 1)):
        wc[:, 9 + j, :] = -(w1c[:, :, e] * w3c[:, 2][:, None]).T

    w4c = np.asarray(w4, dtype=np.float32)[:, :, 0, :]  # (ci, k, g)
    if T4_PACKED:
        w4y = np.zeros((C, 128), dtype=np.float32)
        for k in range(3):
            w4y[:, 32 * k : 32 * k + G] = w4c[:, k, :]
    else:
        w4y = np.ascontiguousarray(np.tile(w4c, (1, 1, C // G)))  # (ci, k, 128)
    sel = np.zeros((128, C), dtype=np.float32)
    for k in range(3):
        for g in range(G):
            sel[32 * k + g, g::G] = 1.0
    f16 = np.float16
    return (
        np.ascontiguousarray(wc).astype(f16),
        np.ascontiguousarray(w4y).astype(f16),
        np.ascontiguousarray(sel).astype(f16),
    )


def kernel(x, w1, w3, w4):
    global LAST_EXEC_NS, LAST_RESULTS
    nc = _get_compiled()
    xh = np.ascontiguousarray(np.asarray(x, dtype=np.float32)).astype(np.float16)
    wc, w4y, sel = _prep_weights(w1, w3, w4)

    in_maps = [
        {
            "x_s": np.ascontiguousarray(xh[i * NPC : (i + 1) * NPC]),
            "wc": wc,
            "w4y": w4y,
            "sel": sel,
        }
        for i in range(NCORES)
    ]
    if TRACE:
        _enable_trace_hook()
    res = bass_utils.run_bass_kernel_spmd(
        nc,
        in_maps,
        core_ids=list(range(NCORES)),
        trace=TRACE,
        tmpdir=TRACE_DIR,
    )
    LAST_EXEC_NS = res.exec_time_ns
    LAST_RESULTS = res
    out = np.concatenate(
        [res.results[i]["out"] for i in range(NCORES)], axis=0
    ).astype(np.float32)
    return out


# revision 12
# speedup vs baseline: 1.0281x; 1.0281x over previous
"""Trainium2 Bass kernel for dense_cnn problem.

Math (per batch element n, C=128 channels, H=W=56, G=8):
  t1 = conv_h(x, w1)          5-tap conv over H with full channel mixing
  t3 = dwconv_h(t1, w3)       3-tap depthwise conv over H
  t4[g] = sum_{c,k} x[c, h, w+2k-2] * w4[c,k,g]   (3 width taps, dil 2)
  out[c] = t3[c] * t4[c % 8]

Device strategy (data-parallel, 4 batch elems per core across 8 cores):
  - Fold t3 = w3 (*) w1 (*) x into ONE 7-tap H-conv with combined weights
    wc[f, ci, co] = sum_{d+e=f} w3[co,d] * w1[co,ci,e]  -> PE matmuls.
    The fold is only exact where the intermediate t1 index stays in
    [0,56); 4 tiny correction matmuls fix output rows 0 and 55.
  - t4 via ONE packed matmul per chunk producing y[32k+g] = sum_c
    x[c]*w4[c,k,g] at partition groups {0-7, 32-39, 64-71}, an ACT copy
    of y to SBUF, then THREE CONCURRENT row-tiled selector matmuls
    (tile_position=(32k,0)) that apply the +/-2 W-shifts and broadcast
    g -> 128 channels while accumulating in one PSUM bank.  Streams
    1x448 + 3x448-concurrent rows instead of 3x448 serial.
  - All matmuls in fp16 (1 cycle/row like bf16 but 10-bit mantissa:
    rel err ~4e-4 vs 2.6e-3); accumulation fp32 in PSUM.
  - Output DMA'd as fp16 (halves write traffic; host upcasts to fp32).
  - Per 8-row chunk: psA (t3), psY (y), psB (t4 broadcast); ScalarE
    copies psY->SBUF and psB->SBUF, VectorE multiplies psA*t4s -> fp16.
  - Startup: GpSimd memsets a dummy tile early; 8 dummy matmuls trip
    the PE_HAM clock gate (1.2 -> 2.4 GHz) while the first DMAs stream;
    x elem 0 lands chunk-first so real matmuls start ~10.3us.
"""

import sys

sys.path.insert(0, "/opt/trn_rl_repo")

import ml_dtypes
import numpy as np

import concourse.bacc as bacc
import concourse.bass as bass
import concourse.mybir as mybir
import concourse.tile as tile
from concourse import bass_utils

N, C, H, W, G = 32, 128, 56, 56, 8
NCORES = 8
NPC = N // NCORES  # batch elems per core
CH = 8             # H rows per chunk
NCHUNK = H // CH

F32 = mybir.dt.float32
F16 = mybir.dt.float16

T4_PACKED = False   # y-matmul + row-tiled broadcast (False: 3 plain taps)
NDUMMY = 7

TRACE = False
TRACE_DIR = None
LAST_EXEC_NS = None
LAST_RESULTS = None

_COMPILED = None


def _enable_trace_hook():
    """The agent image's ``antenv`` lacks ``axon_hooks``, so the boot-time
    NTFF hook registration silently degraded. Recreate the module and
    register the same ctypes-based hook; also skip the bucket upload."""
    import sys as _sys
    import types

    if "antenv.axon_hooks" not in _sys.modules:
        mod = types.ModuleType("antenv.axon_hooks")
        mod._hook = None

        def set_axon_ntff_profile_hook(h):
            mod._hook = h

        def get_axon_ntff_profile_hook():
            return mod._hook

        mod.set_axon_ntff_profile_hook = set_axon_ntff_profile_hook
        mod.get_axon_ntff_profile_hook = get_axon_ntff_profile_hook
        _sys.modules["antenv.axon_hooks"] = mod
        import antenv

        antenv.axon_hooks = mod

    from antenv.axon_hooks import get_axon_ntff_profile_hook as _get

    if _get() is None:
        from trn_agent_boot.trn_boot import _ntff_profile_via_ctypes

        hook = _ntff_profile_via_ctypes("/opt/axon/libaxon_pjrt.so")
        if hook is not None:
            _sys.modules["antenv.axon_hooks"].set_axon_ntff_profile_hook(hook)

    bass_utils.upload_artifacts = lambda tmpdir: f"local:{tmpdir}"


def _t3_matmuls(c, pa, xc, wc_t):
    """(lhsT, rhs, out) list accumulating the folded 7-tap conv for the
    8-row chunk c, with row clipping at the H borders plus the t1-clip
    correction taps. Output row o of the chunk reads x row 8c+o+f-3."""
    h0 = c * CH
    mms = []
    # f=3 covers the full chunk for every c -> emitted first (start=True)
    for f in (3, 0, 1, 2, 4, 5, 6):
        o_lo = max(0, 3 - f - h0)
        o_hi = min(CH, H + 3 - f - h0)
        if o_lo >= o_hi:
            continue
        r0 = h0 + o_lo + f - 3
        r1 = h0 + o_hi + f - 3
        mms.append((wc_t[:, f, :], xc[:, r0:r1, :], pa[:, o_lo:o_hi, :]))
    if c == 0:
        # fold wrongly includes t1[-1] at h=0: subtract w3[0]*w1[e]*x[e-3]
        for j in range(2):
            mms.append((wc_t[:, 7 + j, :], xc[:, j : j + 1, :], pa[:, 0:1, :]))
    if c == NCHUNK - 1:
        # fold wrongly includes t1[56] at h=55
        for j in range(2):
            mms.append(
                (wc_t[:, 9 + j, :], xc[:, 54 + j : 55 + j, :], pa[:, CH - 1 : CH, :])
            )
    return mms


def _t4_matmuls(c, pb, xc, w4_t):
    """Fallback t4: 3 width taps at offsets -2/0/+2, col-clipped."""
    h0 = c * CH
    rows = xc[:, h0 : h0 + CH, :]
    return [
        (w4_t[:, 1, :], rows, pb[:]),                                     # 0
        (w4_t[:, 0, :], xc[:, h0 : h0 + CH, 0 : W - 2], pb[:, :, 2:W]),   # -2
        (w4_t[:, 2, :], xc[:, h0 : h0 + CH, 2:W], pb[:, :, 0 : W - 2]),   # +2
    ]


def _build():
    nc = bacc.Bacc(
        "TRN2",
        target_bir_lowering=False,
        debug=False,
        enable_asserts=False,
        num_devices=NCORES,
    )

    x_d = nc.dram_tensor("x_s", (NPC, C, H, W), F16, kind="ExternalInput").ap()
    wc_d = nc.dram_tensor("wc", (C, 11, C), F16, kind="ExternalInput").ap()
    w4_shape = (C, 128) if T4_PACKED else (C, 3, C)
    w4_d = nc.dram_tensor("w4y", w4_shape, F16, kind="ExternalInput").ap()
    sel_d = nc.dram_tensor("sel", (128, C), F16, kind="ExternalInput").ap()
    out_d = nc.dram_tensor("out", (NPC, C, H, W), F16, kind="ExternalOutput").ap()

    with tile.TileContext(nc) as tc:
        import contextlib
        ctx = contextlib.ExitStack()
        with ctx:
            wpool = ctx.enter_context(tc.tile_pool(name="wpool", bufs=1))
            xpool = ctx.enter_context(tc.tile_pool(name="xpool", bufs=1))
            ypool = ctx.enter_context(tc.tile_pool(name="ypool", bufs=2))
            t4pool = ctx.enter_context(tc.tile_pool(name="t4pool", bufs=3))
            opool = ctx.enter_context(tc.tile_pool(name="opool", bufs=3))
            papool = ctx.enter_context(tc.tile_pool(name="psA", bufs=3, space="PSUM"))
            pbpool = ctx.enter_context(
                tc.tile_pool(name="psB", bufs=2 if T4_PACKED else 3, space="PSUM")
            )
            if T4_PACKED:
                pypool = ctx.enter_context(
                    tc.tile_pool(name="psY", bufs=2, space="PSUM")
                )
            pdpool = ctx.enter_context(tc.tile_pool(name="psD", bufs=1, space="PSUM"))
            # Dummy matmuls while the first DMAs stream in: PE_HAM ungates
            # the 2.4 GHz clock only after ~3.4us of sustained activity.
            # GpSimd does the memset (its preamble finishes earliest).
            dmy = wpool.tile([C, 512], F16)
            nc.gpsimd.memset(dmy[:], 0.0)
            dps = pdpool.tile([C, 512], F32)
            for _ in range(NDUMMY):
                nc.tensor.matmul(
                    dps[:], lhsT=dmy[:, 0:C], rhs=dmy[:], start=True, stop=True
                )

            xcs = []
            for n in range(NPC):
                xc = xpool.tile([C, H, W], F16, name=f"xc{n}")
                xcs.append(xc)

            wc_t = wpool.tile([C, 11, C], F16)
            w4_t = wpool.tile(list(w4_shape), F16)
            sel_t = wpool.tile([128, C], F16)

            # first batch elem lands rows 0..20 first (chunks 0-1 plus
            # their +3 tap halo) so real matmuls start as soon as wc and
            # this piece arrive; the rest streams behind
            nc.sync.dma_start(wc_t[:], wc_d[:])
            nc.sync.dma_start(xcs[0][:, 0:20, :], x_d[0, :, 0:20, :])
            nc.sync.dma_start(w4_t[:], w4_d[:])
            if T4_PACKED:
                nc.sync.dma_start(sel_t[:], sel_d[:])
            nc.sync.dma_start(xcs[0][:, 20:H, :], x_d[0, :, 20:H, :])
            for n in range(1, NPC):
                nc.sync.dma_start(xcs[n][:], x_d[n])

            for n in range(NPC):
                xc = xcs[n]
                ot = None

                for c in range(NCHUNK):
                    h0 = c * CH

                    if T4_PACKED:
                        # y[32k+g, h, w] = sum_c x[c,h,w] w4[c,k,g]
                        py = pypool.tile([C, CH, W], F32)
                        nc.tensor.matmul(
                            py[:], lhsT=w4_t[:], rhs=xc[:, h0 : h0 + CH, :],
                            start=True, stop=True,
                        )

                    pa = papool.tile([C, CH, W], F32)
                    mms = _t3_matmuls(c, pa, xc, wc_t)
                    for i, (lhsT, rhs, outap) in enumerate(mms):
                        nc.tensor.matmul(
                            outap,
                            lhsT=lhsT,
                            rhs=rhs,
                            start=(i == 0),
                            stop=(i == len(mms) - 1),
                        )

                    pb = pbpool.tile([C, CH, W], F32)
                    if T4_PACKED:
                        ys = ypool.tile([C, CH, W], F16)
                        nc.scalar.copy(ys[:], py[:])
                        # three concurrent row-tiled selector matmuls
                        # (K=32, the HW-verified tile config; rows g>=8 of
                        # each group are zero in both sel and y): tile k
                        # contracts y partitions 32k..32k+31, broadcasts
                        # g -> all 128 channels and applies the W-shift
                        nc.tensor.matmul(
                            pb[:], lhsT=sel_t[32:64, :], rhs=ys[32:64, :, :],
                            start=True, stop=False, tile_position=(32, 0),
                        )
                        nc.tensor.matmul(
                            pb[:, :, 2:W], lhsT=sel_t[0:32, :],
                            rhs=ys[0:32, :, 0 : W - 2],
                            start=False, stop=False, tile_position=(0, 0),
                        )
                        nc.tensor.matmul(
                            pb[:, :, 0 : W - 2], lhsT=sel_t[64:96, :],
                            rhs=ys[64:96, :, 2:W],
                            start=False, stop=True, tile_position=(64, 0),
                        )
                    else:
                        mmsb = _t4_matmuls(c, pb, xc, w4_t)
                        for i, (lhsT, rhs, outap) in enumerate(mmsb):
                            nc.tensor.matmul(
                                outap,
                                lhsT=lhsT,
                                rhs=rhs,
                                start=(i == 0),
                                stop=(i == len(mmsb) - 1),
                            )

                    t4s = t4pool.tile([C, CH, W], F16)
                    nc.scalar.copy(t4s[:], pb[:])

                    # output tiles: chunk pairs (0,1),(2,3),(4,5) and a
                    # single-chunk piece for 6, so the final piece after
                    # the last matmul is small and the tail short
                    if c % 2 == 0:
                        rows_t = CH if c == NCHUNK - 1 else 2 * CH
                        ot = opool.tile([C, rows_t, W], F16)
                        ot_c0 = c
                    nc.vector.tensor_mul(
                        ot[:, (c - ot_c0) * CH : (c - ot_c0 + 1) * CH, :],
                        pa[:], t4s[:],
                    )
                    if c % 2 == 1 or c == NCHUNK - 1:
                        rows = (c - ot_c0 + 1) * CH
                        nc.sync.dma_start(
                            out_d[n, :, ot_c0 * CH : ot_c0 * CH + rows, :],
                            ot[:, 0:rows, :],
                        )

    nc.compile()
    return nc


def _get_compiled():
    global _COMPILED
    if _COMPILED is None:
        _COMPILED = _build()
    return _COMPILED


def _prep_weights(w1, w3, w4):
    w1c = np.asarray(w1, dtype=np.float32)[:, :, :, 0]  # (co, ci, 5)
    w3c = np.asarray(w3, dtype=np.float32)[:, 0, :, 0]  # (co, 3)
    wc = np.zeros((C, 11, C), dtype=np.float32)         # (ci, tap, co)
    for d in range(3):
        for e in range(5):
            # wc[ci, d+e, co] += w1[co, ci, e] * w3[co, d]
            wc[:, d + e, :] += (w1c[:, :, e] * w3c[:, d][:, None]).T
    # border clip corrections (see _t3_matmuls): taps 7,8 fix h=0; 9,10 h=55
    for j, e in enumerate((3, 4)):
        wc[:, 7 + j, :] = -(w1c[:, :, e] * w3c[:, 0][:, None]).T
    for j, e in enumerate((0, 1)):
        wc[:, 9 + j, :] = -(w1c[:, :, e] * w3c[:, 2][:, None]).T

    w4c = np.asarray(w4, dtype=np.float32)[:, :, 0, :]  # (ci, k, g)
    if T4_PACKED:
        w4y = np.zeros((C, 128), dtype=np.float32)
        for k in range(3):
            w4y[:, 32 * k : 32 * k + G] = w4c[:, k, :]
    else:
        w4y = np.ascontiguousarray(np.tile(w4c, (1, 1, C // G)))  # (ci, k, 128)
    sel = np.zeros((128, C), dtype=np.float32)
    for k in range(3):
        for g in range(G):
            sel[32 * k + g, g::G] = 1.0
    f16 = np.float16
    return (
        np.ascontiguousarray(wc).astype(f16),
        np.ascontiguousarray(w4y).astype(f16),
        np.ascontiguousarray(sel).astype(f16),
    )


def kernel(x, w1, w3, w4):
    global LAST_EXEC_NS, LAST_RESULTS
    nc = _get_compiled()
    xh = np.ascontiguousarray(np.asarray(x, dtype=np.float32)).astype(np.float16)
    wc, w4y, sel = _prep_weights(w1, w3, w4)

    in_maps = [
        {
            "x_s": np.ascontiguousarray(xh[i * NPC : (i + 1) * NPC]),
            "wc": wc,
            "w4y": w4y,
            "sel": sel,
        }
        for i in range(NCORES)
    ]
    if TRACE:
        _enable_trace_hook()
    res = bass_utils.run_bass_kernel_spmd(
        nc,
        in_maps,
        core_ids=list(range(NCORES)),
        trace=TRACE,
        tmpdir=TRACE_DIR,
    )
    LAST_EXEC_NS = res.exec_time_ns
    LAST_RESULTS = res
    out = np.concatenate(
        [res.results[i]["out"] for i in range(NCORES)], axis=0
    ).astype(np.float32)
    return out


# revision 16
# speedup vs baseline: 1.0817x; 1.0521x over previous
"""Trainium2 Bass kernel for dense_cnn problem.

Math (per batch element n, C=128 channels, H=W=56, G=8):
  t1 = conv_h(x, w1)          5-tap conv over H with full channel mixing
  t3 = dwconv_h(t1, w3)       3-tap depthwise conv over H
  t4[g] = sum_{c,k} x[c, h, w+2k-2] * w4[c,k,g]   (3 width taps, dil 2)
  out[c] = t3[c] * t4[c % 8]

Device strategy (data-parallel, 4 batch elems per core across 8 cores):
  - Fold t3 = w3 (*) w1 (*) x into ONE 7-tap H-conv with combined weights
    wc[f, ci, co] = sum_{d+e=f} w3[co,d] * w1[co,ci,e]  -> PE matmuls.
    The fold is only exact where the intermediate t1 index stays in
    [0,56); 4 tiny correction matmuls fix output rows 0 and 55.
  - t4 via ONE packed matmul per chunk producing y[32k+g] = sum_c
    x[c]*w4[c,k,g] at partition groups {0-7, 32-39, 64-71}, an ACT copy
    of y to SBUF, then THREE CONCURRENT row-tiled selector matmuls
    (tile_position=(32k,0)) that apply the +/-2 W-shifts and broadcast
    g -> 128 channels while accumulating in one PSUM bank.  Streams
    1x448 + 3x448-concurrent rows instead of 3x448 serial.
  - All matmuls in fp16 (1 cycle/row like bf16 but 10-bit mantissa:
    rel err ~4e-4 vs 2.6e-3); accumulation fp32 in PSUM.
  - Output DMA'd as fp16 (halves write traffic; host upcasts to fp32).
  - Per 8-row chunk: psA (t3), psY (y), psB (t4 broadcast); ScalarE
    copies psY->SBUF and psB->SBUF, VectorE multiplies psA*t4s -> fp16.
  - Startup: GpSimd memsets a dummy tile early; 8 dummy matmuls trip
    the PE_HAM clock gate (1.2 -> 2.4 GHz) while the first DMAs stream;
    x elem 0 lands chunk-first so real matmuls start ~10.3us.
"""

import sys

sys.path.insert(0, "/opt/trn_rl_repo")

import ml_dtypes
import numpy as np

import concourse.bacc as bacc
import concourse.bass as bass
import concourse.mybir as mybir
import concourse.tile as tile
from concourse import bass_utils

N, C, H, W, G = 32, 128, 56, 56, 8
NCORES = 8
NPC = N // NCORES  # batch elems per core
CH = 8             # H rows per chunk
NCHUNK = H // CH

F32 = mybir.dt.float32
F16 = mybir.dt.float16

T4_PACKED = False   # y-matmul + row-tiled broadcast (False: 3 plain taps)
NDUMMY = 7

TRACE = False
TRACE_DIR = None
LAST_EXEC_NS = None
LAST_RESULTS = None

_COMPILED = None


def _enable_trace_hook():
    """The agent image's ``antenv`` lacks ``axon_hooks``, so the boot-time
    NTFF hook registration silently degraded. Recreate the module and
    register the same ctypes-based hook; also skip the bucket upload."""
    import sys as _sys
    import types

    if "antenv.axon_hooks" not in _sys.modules:
        mod = types.ModuleType("antenv.axon_hooks")
        mod._hook = None

        def set_axon_ntff_profile_hook(h):
            mod._hook = h

        def get_axon_ntff_profile_hook():
            return mod._hook

        mod.set_axon_ntff_profile_hook = set_axon_ntff_profile_hook
        mod.get_axon_ntff_profile_hook = get_axon_ntff_profile_hook
        _sys.modules["antenv.axon_hooks"] = mod
        import antenv

        antenv.axon_hooks = mod

    from antenv.axon_hooks import get_axon_ntff_profile_hook as _get

    if _get() is None:
        from trn_agent_boot.trn_boot import _ntff_profile_via_ctypes

        hook = _ntff_profile_via_ctypes("/opt/axon/libaxon_pjrt.so")
        if hook is not None:
            _sys.modules["antenv.axon_hooks"].set_axon_ntff_profile_hook(hook)

    bass_utils.upload_artifacts = lambda tmpdir: f"local:{tmpdir}"


# taps live in DRAM/SBUF in this order so that the prefix WCA_TAPS (all
# chunk 0 needs for its first 6 matmuls) can be DMA'd as a first piece
TAP_ORDER = (3, 0, 1, 2, 7, 8, 4, 5, 6, 9, 10)
TAP_SLOT = {f: i for i, f in enumerate(TAP_ORDER)}
WCA_NTAPS = 6  # slots 0..5 = taps {3,0,1,2,7,8}


def _t3_matmuls(c, pa, xc, wc_t):
    """(lhsT, rhs, out) list accumulating the folded 7-tap conv for the
    8-row chunk c, with row clipping at the H borders plus the t1-clip
    correction taps. Output row o of the chunk reads x row 8c+o+f-3.
    For chunk 0 the taps stored in the first wc DMA piece come first."""
    h0 = c * CH
    order = (3, 0, 1, 2, 7, 8, 4, 5, 6) if c == 0 else (3, 0, 1, 2, 4, 5, 6)
    mms = []
    for f in order:
        if f >= 7:
            # fold wrongly includes t1[-1] at h=0: -w3[0]*w1[e]*x[e-3]
            j = f - 7
            mms.append(
                (wc_t[:, TAP_SLOT[f], :], xc[:, j : j + 1, :], pa[:, 0:1, :])
            )
            continue
        o_lo = max(0, 3 - f - h0)
        o_hi = min(CH, H + 3 - f - h0)
        if o_lo >= o_hi:
            continue
        r0 = h0 + o_lo + f - 3
        r1 = h0 + o_hi + f - 3
        mms.append(
            (wc_t[:, TAP_SLOT[f], :], xc[:, r0:r1, :], pa[:, o_lo:o_hi, :])
        )
    if c == NCHUNK - 1:
        # fold wrongly includes t1[56] at h=55
        for j in range(2):
            mms.append(
                (
                    wc_t[:, TAP_SLOT[9 + j], :],
                    xc[:, 54 + j : 55 + j, :],
                    pa[:, CH - 1 : CH, :],
                )
            )
    return mms


def _t4_matmuls(c, pb, xc, w4_t):
    """Fallback t4: 3 width taps at offsets -2/0/+2, col-clipped."""
    h0 = c * CH
    rows = xc[:, h0 : h0 + CH, :]
    return [
        (w4_t[:, 1, :], rows, pb[:]),                                     # 0
        (w4_t[:, 0, :], xc[:, h0 : h0 + CH, 0 : W - 2], pb[:, :, 2:W]),   # -2
        (w4_t[:, 2, :], xc[:, h0 : h0 + CH, 2:W], pb[:, :, 0 : W - 2]),   # +2
    ]


def _build():
    nc = bacc.Bacc(
        "TRN2",
        target_bir_lowering=False,
        debug=False,
        enable_asserts=False,
        num_devices=NCORES,
    )

    x_d = nc.dram_tensor("x_s", (NPC, C, H, W), F16, kind="ExternalInput").ap()
    wc_d = nc.dram_tensor("wc", (C, 11, C), F16, kind="ExternalInput").ap()
    w4_shape = (C, 128) if T4_PACKED else (C, 3, C)
    w4_d = nc.dram_tensor("w4y", w4_shape, F16, kind="ExternalInput").ap()
    sel_d = nc.dram_tensor("sel", (128, C), F16, kind="ExternalInput").ap()
    out_d = nc.dram_tensor("out", (NPC, C, H, W), F16, kind="ExternalOutput").ap()

    with tile.TileContext(nc) as tc:
        import contextlib
        ctx = contextlib.ExitStack()
        with ctx:
            wpool = ctx.enter_context(tc.tile_pool(name="wpool", bufs=1))
            xpool = ctx.enter_context(tc.tile_pool(name="xpool", bufs=1))
            ypool = ctx.enter_context(tc.tile_pool(name="ypool", bufs=2))
            t4pool = ctx.enter_context(tc.tile_pool(name="t4pool", bufs=3))
            opool = ctx.enter_context(tc.tile_pool(name="opool", bufs=3))
            papool = ctx.enter_context(tc.tile_pool(name="psA", bufs=3, space="PSUM"))
            pbpool = ctx.enter_context(
                tc.tile_pool(name="psB", bufs=2 if T4_PACKED else 3, space="PSUM")
            )
            if T4_PACKED:
                pypool = ctx.enter_context(
                    tc.tile_pool(name="psY", bufs=2, space="PSUM")
                )
            pdpool = ctx.enter_context(tc.tile_pool(name="psD", bufs=1, space="PSUM"))
            # Dummy matmuls while the first DMAs stream in: PE_HAM ungates
            # the 2.4 GHz clock only after ~3.4us of sustained activity.
            # GpSimd does the memset (its preamble finishes earliest).
            dmy = wpool.tile([C, 512], F16)
            nc.gpsimd.memset(dmy[:], 0.0)
            dps = pdpool.tile([C, 512], F32)
            for _ in range(NDUMMY):
                nc.tensor.matmul(
                    dps[:], lhsT=dmy[:, 0:C], rhs=dmy[:], start=True, stop=True
                )

            xcs = []
            for n in range(NPC):
                xc = xpool.tile([C, H, W], F16, name=f"xc{n}")
                xcs.append(xc)

            wc_t = wpool.tile([C, 11, C], F16)
            w4_t = wpool.tile(list(w4_shape), F16)
            sel_t = wpool.tile([128, C], F16)

            # DMA pieces ordered by when the PE needs them (in-DMA
            # sustains ~250 GB/s with the first packet ~8.1us): the wc
            # prefix and x rows 0..16 gate the first real matmul ~9.6us
            nc.sync.dma_start(wc_t[:, 0:WCA_NTAPS, :], wc_d[:, 0:WCA_NTAPS, :])
            nc.sync.dma_start(xcs[0][:, 0:16, :], x_d[0, :, 0:16, :])
            nc.sync.dma_start(wc_t[:, WCA_NTAPS:11, :], wc_d[:, WCA_NTAPS:11, :])
            nc.sync.dma_start(w4_t[:], w4_d[:])
            if T4_PACKED:
                nc.sync.dma_start(sel_t[:], sel_d[:])
            nc.sync.dma_start(xcs[0][:, 16:28, :], x_d[0, :, 16:28, :])
            nc.sync.dma_start(xcs[0][:, 28:H, :], x_d[0, :, 28:H, :])
            for n in range(1, NPC):
                nc.sync.dma_start(xcs[n][:], x_d[n])

            for n in range(NPC):
                xc = xcs[n]
                ot = None

                for c in range(NCHUNK):
                    h0 = c * CH

                    if T4_PACKED:
                        # y[32k+g, h, w] = sum_c x[c,h,w] w4[c,k,g]
                        py = pypool.tile([C, CH, W], F32)
                        nc.tensor.matmul(
                            py[:], lhsT=w4_t[:], rhs=xc[:, h0 : h0 + CH, :],
                            start=True, stop=True,
                        )

                    pa = papool.tile([C, CH, W], F32)
                    mms = _t3_matmuls(c, pa, xc, wc_t)
                    for i, (lhsT, rhs, outap) in enumerate(mms):
                        nc.tensor.matmul(
                            outap,
                            lhsT=lhsT,
                            rhs=rhs,
                            start=(i == 0),
                            stop=(i == len(mms) - 1),
                        )

                    pb = pbpool.tile([C, CH, W], F32)
                    if T4_PACKED:
                        ys = ypool.tile([C, CH, W], F16)
                        nc.scalar.copy(ys[:], py[:])
                        # three concurrent row-tiled selector matmuls
                        # (K=32, the HW-verified tile config; rows g>=8 of
                        # each group are zero in both sel and y): tile k
                        # contracts y partitions 32k..32k+31, broadcasts
                        # g -> all 128 channels and applies the W-shift
                        nc.tensor.matmul(
                            pb[:], lhsT=sel_t[32:64, :], rhs=ys[32:64, :, :],
                            start=True, stop=False, tile_position=(32, 0),
                        )
                        nc.tensor.matmul(
                            pb[:, :, 2:W], lhsT=sel_t[0:32, :],
                            rhs=ys[0:32, :, 0 : W - 2],
                            start=False, stop=False, tile_position=(0, 0),
                        )
                        nc.tensor.matmul(
                            pb[:, :, 0 : W - 2], lhsT=sel_t[64:96, :],
                            rhs=ys[64:96, :, 2:W],
                            start=False, stop=True, tile_position=(64, 0),
                        )
                    else:
                        mmsb = _t4_matmuls(c, pb, xc, w4_t)
                        for i, (lhsT, rhs, outap) in enumerate(mmsb):
                            nc.tensor.matmul(
                                outap,
                                lhsT=lhsT,
                                rhs=rhs,
                                start=(i == 0),
                                stop=(i == len(mmsb) - 1),
                            )

                    if c == NCHUNK - 1:
                        # last chunk: copy/multiply/DMA in two 4-row
                        # pieces so the post-matmul tail pipelines
                        hh = CH // 2
                        for p in range(2):
                            t4s = t4pool.tile([C, hh, W], F16)
                            nc.scalar.copy(
                                t4s[:], pb[:, p * hh : (p + 1) * hh, :]
                            )
                            ot = opool.tile([C, hh, W], F16)
                            nc.vector.tensor_mul(
                                ot[:], pa[:, p * hh : (p + 1) * hh, :], t4s[:]
                            )
                            r0 = h0 + p * hh
                            nc.sync.dma_start(
                                out_d[n, :, r0 : r0 + hh, :], ot[:]
                            )
                        continue

                    t4s = t4pool.tile([C, CH, W], F16)
                    nc.scalar.copy(t4s[:], pb[:])

                    # output tiles: chunk pairs (0,1),(2,3),(4,5)
                    if c % 2 == 0:
                        ot = opool.tile([C, 2 * CH, W], F16)
                        ot_c0 = c
                    nc.vector.tensor_mul(
                        ot[:, (c - ot_c0) * CH : (c - ot_c0 + 1) * CH, :],
                        pa[:], t4s[:],
                    )
                    if c % 2 == 1:
                        rows = (c - ot_c0 + 1) * CH
                        nc.sync.dma_start(
                            out_d[n, :, ot_c0 * CH : ot_c0 * CH + rows, :],
                            ot[:, 0:rows, :],
                        )

    nc.compile()
    return nc


def _get_compiled():
    global _COMPILED
    if _COMPILED is None:
        _COMPILED = _build()
    return _COMPILED


def _prep_weights(w1, w3, w4):
    w1c = np.asarray(w1, dtype=np.float32)[:, :, :, 0]  # (co, ci, 5)
    w3c = np.asarray(w3, dtype=np.float32)[:, 0, :, 0]  # (co, 3)
    wc = np.zeros((C, 11, C), dtype=np.float32)         # (ci, tap, co)
    for d in range(3):
        for e in range(5):
            # wc[ci, d+e, co] += w1[co, ci, e] * w3[co, d]
            wc[:, d + e, :] += (w1c[:, :, e] * w3c[:, d][:, None]).T
    # border clip corrections (see _t3_matmuls): taps 7,8 fix h=0; 9,10 h=55
    for j, e in enumerate((3, 4)):
        wc[:, 7 + j, :] = -(w1c[:, :, e] * w3c[:, 0][:, None]).T
    for j, e in enumerate((0, 1)):
        wc[:, 9 + j, :] = -(w1c[:, :, e] * w3c[:, 2][:, None]).T
    wc = wc[:, list(TAP_ORDER), :]  # DMA-piece order; kernel uses TAP_SLOT

    w4c = np.asarray(w4, dtype=np.float32)[:, :, 0, :]  # (ci, k, g)
    if T4_PACKED:
        w4y = np.zeros((C, 128), dtype=np.float32)
        for k in range(3):
            w4y[:, 32 * k : 32 * k + G] = w4c[:, k, :]
    else:
        w4y = np.ascontiguousarray(np.tile(w4c, (1, 1, C // G)))  # (ci, k, 128)
    sel = np.zeros((128, C), dtype=np.float32)
    for k in range(3):
        for g in range(G):
            sel[32 * k + g, g::G] = 1.0
    f16 = np.float16
    return (
        np.ascontiguousarray(wc).astype(f16),
        np.ascontiguousarray(w4y).astype(f16),
        np.ascontiguousarray(sel).astype(f16),
    )


def kernel(x, w1, w3, w4):
    global LAST_EXEC_NS, LAST_RESULTS
    nc = _get_compiled()
    xh = np.ascontiguousarray(np.asarray(x, dtype=np.float32)).astype(np.float16)
    wc, w4y, sel = _prep_weights(w1, w3, w4)

    in_maps = [
        {
            "x_s": np.ascontiguousarray(xh[i * NPC : (i + 1) * NPC]),
            "wc": wc,
            "w4y": w4y,
            "sel": sel,
        }
        for i in range(NCORES)
    ]
    if TRACE:
        _enable_trace_hook()
    res = bass_utils.run_bass_kernel_spmd(
        nc,
        in_maps,
        core_ids=list(range(NCORES)),
        trace=TRACE,
        tmpdir=TRACE_DIR,
    )
    LAST_EXEC_NS = res.exec_time_ns
    LAST_RESULTS = res
    out = np.concatenate(
        [res.results[i]["out"] for i in range(NCORES)], axis=0
    ).astype(np.float32)
    return out
